# revision 23
# baseline (speedup 1.0000x reference)
"""MetaNCA Trainium2 kernel: out = softmax(X @ (W + MLP_percell(W))).

Strategy (8 NeuronCores, SPMD, fp16 matmuls — 1 cyc/row on HW vs 2 for f32r):
  - W row-sharded (256 rows/core) as 126-row tiles; partition 126/127 of each
    tile hold the colsum row / ones row so the whole first MLP layer is ONE
    K=128 matmul per sub-chunk. Global colsum/rowsum baked in on host.
  - MLP (3->10->10->1) as block-diag fp16 matmuls, 12 rows/chain, software
    pipelined 3 deep, column-slice outer (j of 512 cols); each finished slice
    is staged and AllGathered immediately (4 column AGs overlap MLP+phase 3).
    AG outputs land in Shared-addr-space DRAM (fast HBM-HBM collective path).
  - Phase 3: per column pass p, per X row-block bt, accumulate [128,512]
    logits over 16 k-tiles in one PSUM bank (8 banks = 8 row-blocks in
    flight); then ONLINE softmax: rowwise max of the pass slice, exp(l-max_p)
    written fp16 straight from PSUM, DMA'd out. Per-pass maxes are returned
    in a side tensor; host rescales slices by exp(max_p - max_row) and
    normalizes. X^T is host-pre-arranged fp16 per-core as [bt][p][kt*128].
"""

import os
import sys

import numpy as np

for _p in ("/opt/trn_rl_repo", "/root/.axon_site/_ro/trn_rl_repo"):
    if os.path.isdir(_p) and _p not in sys.path:
        sys.path.insert(0, _p)

import concourse.bass as bass  # noqa: E402
import concourse.tile as tile  # noqa: E402
from concourse import bacc, bass_utils, mybir  # noqa: E402

F32 = mybir.dt.float32
F16 = mybir.dt.float16
BF16 = mybir.dt.bfloat16
F32R = mybir.dt.float32r
AF = mybir.ActivationFunctionType
Alu = mybir.AluOpType
H = 10
RW = 126  # real W rows per tile (126/127 = colsum/ones)
# tile-scheduler pseudo-time hints (ms) for phase-3 weight loads
P3_W0 = 0.055
P3_WP = 0.020


def _tile_plan(n_shard):
    plan = []
    r = 0
    while r + RW <= n_shard:
        plan.append(RW)
        r += RW
    if r < n_shard:
        plan.append(n_shard - r)
    return plan


def _subchunks(rows):
    subs = []
    r = 0
    while r < rows:
        g = min(12, rows - r)
        subs.append((r, g))
        r += g
    return subs


def _agmap(n_shard, plan):
    """ag_in row -> local shard row: tile boundaries placed so [0:128) is
    covered by tiles {0, last} and [128:256) by {1, last}."""
    assert n_shard == 256 and plan == [126, 126, 4]
    t0 = list(range(0, 126))
    t1 = list(range(126, 252))
    t2 = list(range(252, 256))
    return t0 + t2[0:2] + t1 + t2[2:4]


def build_consts(W1, b1, W2, b2, W3, n, m, n_shard):
    alpha = (W1[0] - W1[1] / np.float32(n - 1) - W1[2] / np.float32(m - 1)).astype(np.float32)
    beta = (W1[1] / np.float32(n - 1)).astype(np.float32)
    gamma = (W1[2] / np.float32(m - 1)).astype(np.float32)
    plan = _tile_plan(n_shard)

    def selb(rows):
        cols = rows * H
        t = np.zeros((128, cols), dtype=np.float32)
        for r in range(rows):
            t[r, r * H : (r + 1) * H] = alpha
        t[126, :] = np.tile(beta, rows)
        return t

    def w3sc(rows):
        subs = _subchunks(rows)
        t = np.zeros((120, len(subs) * 128), dtype=np.float32)
        for s, (r0, g_) in enumerate(subs):
            for g in range(g_):
                t[g * H : (g + 1) * H, s * 128 + r0 + g] = W3[:, 0]
        return t

    def blkdiag(mat, g_):
        out = np.zeros((g_ * mat.shape[0], g_ * mat.shape[1]), dtype=np.float32)
        for g in range(g_):
            out[g * mat.shape[0] : (g + 1) * mat.shape[0],
                g * mat.shape[1] : (g + 1) * mat.shape[1]] = mat
        return out

    gset = sorted({g for rows in set(plan) for _, g in _subchunks(rows)})
    c = {}
    for rows in sorted(set(plan)):
        c[f"w3sc{rows}"] = w3sc(rows).astype(np.float16)
    for g_ in gset:
        c[f"w2b{g_}"] = blkdiag(W2, g_).astype(np.float16)
        c[f"b2t{g_}"] = np.tile(b2, g_)[:, None].astype(np.float32)
    c["_selb"] = {rows: selb(rows) for rows in sorted(set(plan))}
    c["_gamma"] = gamma
    c["_b1"] = b1
    return c


def build_program(B, N, M, n_cores):
    n_shard = N // n_cores
    b_shard = B // n_cores
    plan = _tile_plan(n_shard)
    nt = len(plan)
    kt_all = N // 128
    nck_all = N // 127 + (1 if N % 127 else 0)  # phase-3 K-chunks of <=127
    bt_all = b_shard // 128
    jt = M // 512
    tiles_order = [0, nt - 1] + list(range(1, nt - 1)) if nt > 2 else list(range(nt))

    nc = bacc.Bacc("TRN2", target_bir_lowering=False, debug=False, num_devices=n_cores)

    d = {}
    def din(name, shape, dt):
        d[name] = nc.dram_tensor(name, list(shape), dt, kind="ExternalInput").ap()
    din("wsh", (nt, 128, M), F16)              # W shard tiles; row126=colsum, row127=1
    din("xtc", (bt_all, 128, nck_all * 128), F16)  # X^T, [bt][k-in-chunk][ck*128+b]
    for ti, rows in enumerate(plan):
        din(f"selbT{ti}", (128, rows * H), F16)   # alpha diag, row126=beta, row127=r13
    for rows in sorted(set(plan)):
        din(f"w3sc{rows}", (120, len(_subchunks(rows)) * 128), F16)
    gset = sorted({g for rows in set(plan) for _, g in _subchunks(rows)})
    for g_ in gset:
        din(f"w2b{g_}", (g_ * H, g_ * H), F16)
        din(f"b2t{g_}", (g_ * H, 1), F32)
    out_d = nc.dram_tensor("out", [b_shard, M], F16, kind="ExternalOutput").ap()
    mxo_d = nc.dram_tensor("mxo", [128, bt_all * jt], F32, kind="ExternalOutput").ap()

    rg = [list(range(n_cores))]
    agmap = _agmap(n_shard, plan) if n_shard == 256 else list(range(n_shard))

    # contiguous runs (ag_row, tile, local_row, count) of the agmap
    tile_base = []
    acc = 0
    for rows in plan:
        tile_base.append(acc)
        acc += rows
    def tile_of(shard_row):
        for t in range(len(plan) - 1, -1, -1):
            if shard_row >= tile_base[t]:
                return t, shard_row - tile_base[t]
        raise AssertionError
    def ag_runs():
        runs = []
        i = 0
        while i < n_shard:
            t0_, lr0 = tile_of(agmap[i])
            j = i
            while j + 1 < n_shard:
                t1_, lr1 = tile_of(agmap[j + 1])
                if t1_ != t0_ or lr1 != lr0 + (j + 1 - i):
                    break
                j += 1
            runs.append((i, t0_, lr0, j - i + 1))
            i = j + 1
        return runs
    runs = ag_runs()

    with tc_ctx(nc) as tc:
      with tc.tile_pool(name="dram", bufs=1, space="DRAM") as dram:
        ag_in = [dram.tile([n_shard, 512], F16, name=f"ag_in{j}") for j in range(jt)]
        ag_space = "Shared" if n_cores > 4 else "Local"
        agS = [dram.tile([n_cores * n_shard, 512], F16, name=f"agS{j}",
                         addr_space=ag_space) for j in range(jt)]
        with tc.tile_pool(name="xp", bufs=1) as xp:
          with tc.tile_pool(name="cp", bufs=1) as cp, \
               tc.tile_pool(name="wp", bufs=1) as wp:
            def load(pool, name, dram_ap, shape, dt, eng=None):
                t = pool.tile(shape, dt, name=name)
                (eng or nc.scalar).dma_start(t[:], dram_ap[:])
                return t

            # MLP-critical loads on sync, ordered by first use; scalar ring
            # stays clear so the relu stream starts immediately
            w_t, selbw = [None] * nt, [None] * nt
            w3_t, w2b_t, b2t_t = {}, {}, {}
            def load_tile_pair(ti, eng, sliced=False):
                rows = plan[ti]
                t = wp.tile([128, M], F16, name=f"w_t{ti}")
                st = wp.tile([128, rows * H], F16, name=f"selbw{ti}")
                eng.dma_start(st[:], d[f"selbT{ti}"][:])
                if sliced:
                    # j-slice the first tile's load so mm1 of (j=0, ti) can
                    # start as soon as its first 512 columns land
                    for j in range(jt):
                        sl = slice(j * 512, (j + 1) * 512)
                        eng.dma_start(t[:, sl], d["wsh"][ti, :, sl])
                else:
                    eng.dma_start(t[:], d["wsh"][ti])
                w_t[ti], selbw[ti] = t, st
            load_tile_pair(tiles_order[0], nc.sync, sliced=True)
            load_tile_pair(tiles_order[-1], nc.scalar, sliced=True)
            for g_ in gset:
                w2b_t[g_] = load(cp, f"w2b_t{g_}", d[f"w2b{g_}"],
                                 [g_ * H, g_ * H], F16, eng=nc.sync)
                b2t_t[g_] = load(cp, f"b2t_t{g_}", d[f"b2t{g_}"], [g_ * H, 1], F32,
                                 eng=nc.sync)
            for rows in sorted(set(plan), reverse=True):
                w3_t[rows] = load(cp, f"w3_t{rows}", d[f"w3sc{rows}"],
                                  [120, len(_subchunks(rows)) * 128], F16,
                                  eng=nc.sync)
            for ti in tiles_order[1:-1]:
                load_tile_pair(ti, nc.sync)
            # X prefetch after the critical loads (sync ring; needed ~t+60us,
            # must NOT sit on the scalar ring ahead of the MLP relu stream)
            xtb = []
            for bt in range(bt_all):
                t = xp.tile([128, nck_all * 128], F16, name=f"xtb{bt}")
                nc.sync.dma_start(t[:], d["xtc"][bt])
                xtb.append(t)

            with tc.tile_pool(name="p1", bufs=1) as p1:
                # warm exp + relu activation tables before the relu stream
                # needs them (each cold table load costs ~2.7us on scalar)
                wdum = p1.tile([1, 8], F32, name="wdum")
                nc.vector.memset(wdum[:], 0.0)
                nc.scalar.activation(wdum[:], wdum[:], AF.Exp)
                nc.scalar.activation(wdum[:], wdum[:], AF.Relu)

            # ---- phase 2: MLP, column-slice outer; AG per column slice
            with tc.tile_pool(name="nwp", bufs=1) as nwp, \
                 tc.tile_pool(name="hp", bufs=3) as hp, \
                 tc.tile_pool(name="p2ps", bufs=1, space="PSUM") as p2ps:
                chains = []
                for j in range(jt):
                    for ti in tiles_order:
                        rows = plan[ti]
                        for s, (r0, g_) in enumerate(_subchunks(rows)):
                            chains.append((ti, j, s, r0, g_, rows))
                nC = len(chains)
                state = {}
                nw_t = {}
                # greedy scalar/vector balance for the PSUM-source elementwise
                # stream (both engines are ~1x from PSUM; vector also owns the
                # nw adds)
                ew_ns = {"s": 0.0, "v": 0.0}

                def pick_ew():
                    k = "s" if ew_ns["s"] <= ew_ns["v"] else "v"
                    ew_ns[k] += 690.0
                    return k

                def emit_mm1(c, idx):
                    ti, j, s, r0, g_, rows = c
                    sl = slice(j * 512, (j + 1) * 512)
                    Mh = g_ * H
                    ps1 = p2ps.tile([120, 512], F32, name=f"ps1_{ti}_{j}_{s}", tag="ps1", bufs=3)
                    nc.tensor.matmul(ps1[0:Mh, :], selbw[ti][:, r0 * H : r0 * H + Mh],
                                     w_t[ti][:, sl], start=True, stop=True)
                    h1 = hp.tile([120, 512], F16, name=f"h1_{ti}_{j}_{s}", tag="h1")
                    if pick_ew() == "s":
                        nc.scalar.activation(h1[0:Mh, :], ps1[0:Mh, :], AF.Relu)
                    else:
                        nc.vector.tensor_scalar(h1[0:Mh, :], ps1[0:Mh, :], 0.0,
                                                0.0, op0=Alu.max, op1=Alu.bypass)
                    state[c] = (ps1, h1)

                def emit_mm2(c, idx):
                    ti, j, s, r0, g_, rows = c
                    Mh = g_ * H
                    _, h1 = state[c]
                    ps2 = p2ps.tile([120, 512], F32, name=f"ps2_{ti}_{j}_{s}", tag="ps2", bufs=3)
                    nc.tensor.matmul(ps2[0:Mh, :], w2b_t[g_][:], h1[0:Mh, :], start=True, stop=True)
                    h2 = hp.tile([120, 512], F16, name=f"h2_{ti}_{j}_{s}", tag="h2")
                    if pick_ew() == "v":
                        nc.vector.tensor_scalar(h2[0:Mh, :], ps2[0:Mh, :], b2t_t[g_][0:Mh, :],
                                                0.0, op0=Alu.add, op1=Alu.max)
                    else:
                        nc.scalar.activation(h2[0:Mh, :], ps2[0:Mh, :], AF.Relu,
                                             bias=b2t_t[g_][0:Mh, :])
                    state[c] = (state[c][0], state[c][1], ps2, h2)

                def emit_mm3(c):
                    ti, j, s, r0, g_, rows = c
                    sl = slice(j * 512, (j + 1) * 512)
                    Mh = g_ * H
                    h2 = state.pop(c)[3]
                    subs = _subchunks(rows)
                    key = (ti, j)
                    if key not in upd_ps:
                        upd_ps[key] = p2ps.tile([128, 512], F32, name=f"upd_{ti}_{j}",
                                                tag="upd", bufs=2)
                    nc.tensor.matmul(upd_ps[key][:], w3_t[rows][0:Mh, s * 128 : (s + 1) * 128],
                                     h2[0:Mh, :], start=(s == 0), stop=(s == len(subs) - 1))
                    if s == len(subs) - 1:
                        if ti not in nw_t:
                            nw_t[ti] = nwp.tile([128, M], F16, name=f"nw_t{ti}", tag=f"nw{ti}")
                        nc.vector.tensor_tensor(nw_t[ti][0:rows, sl], upd_ps[key][0:rows, :],
                                                w_t[ti][0:rows, sl], op=Alu.add)
                        ew_ns["v"] += 690.0
                        del upd_ps[key]
                        # stage this tile's rows of column slice j into ag_in[j]
                        for (agr, ti2, lr, cnt) in runs:
                            if ti2 != ti:
                                continue
                            nc.gpsimd.dma_start(ag_in[j][agr : agr + cnt, :],
                                              nw_t[ti][lr : lr + cnt, sl])
                        done_tiles[j].add(ti)
                        if len(done_tiles[j]) == nt and not agd.get(j):
                            agd[j] = True
                            nc.gpsimd.collective_compute(
                                "AllGather", Alu.bypass, ins=[ag_in[j].opt()],
                                outs=[agS[j].opt()], replica_groups=rg)

                upd_ps, agd = {}, {}
                done_tiles = {j: set() for j in range(jt)}
                D2, D3 = 2, 4
                for i in range(nC + D3):
                    if i < nC:
                        emit_mm1(chains[i], i)
                    if 0 <= i - D2 < nC:
                        emit_mm2(chains[i - D2], i)
                    if 0 <= i - D3 < nC:
                        emit_mm3(chains[i - D3])

          # ---- phase 3: per column pass, accumulate logits over K-chunks of
          # 127 (a full 128x128 stationary takes the slow weight path on HW:
          # 263ns/mm vs ~220 when either dim <128, so 17 chunks of <=127 beat
          # 16 of 128). Online softmax per (pass, row-block).
          n_main = N // 127          # main K-chunks of 127 rows
          ktail = N - n_main * 127   # remainder K-chunk
          n_ck = n_main + (1 if ktail else 0)
          assert n_main % 4 == 0
          kqh = n_main // 4          # main chunks per quarter-load
          with tc.tile_pool(name="wnp", bufs=1) as wnp, \
               tc.tile_pool(name="smp", bufs=2) as smp, \
               tc.tile_pool(name="mxp", bufs=1) as mxp, \
               tc.tile_pool(name="p3ps", bufs=1, space="PSUM") as p3ps:
              mxall = mxp.tile([128, bt_all * jt], F32, name="mxall")
              for p in range(jt):
                  tc.tile_set_cur_wait(P3_W0 + P3_WP * p)
                  # quarter loads on sync+gpsimd rings (parallel transfers);
                  # the scalar ring is reserved for the exp/out stream so
                  # matmul-feeding loads never queue behind it
                  wnh = []
                  for hf in range(4):
                      t16 = wnp.tile([128, kqh * 512], F16, name=f"wn16_{p}_{hf}",
                                     tag="wn16", bufs=8)
                      src = agS[p][hf * kqh * 127 : (hf + 1) * kqh * 127, :] \
                          .rearrange("(t q) m -> q t m", q=127)
                      eng = nc.sync if hf % 2 == 0 else nc.gpsimd
                      eng.dma_start(t16[0:127, :].rearrange("q (t m) -> q t m", m=512),
                                    src)
                      wnh.append(t16)
                  wtail = None
                  if ktail:
                      wtail = wnp.tile([ktail, 512], F16, name=f"wntail_{p}",
                                       tag="wntail", bufs=2)
                      nc.sync.dma_start(wtail[:], agS[p][n_main * 127 :, :])
                  for bt in range(bt_all):
                      lg = p3ps.tile([128, 512], F32, name=f"lg{p}_{bt}", tag="lg",
                                     bufs=min(8, bt_all))
                      for c in range(n_ck):
                          if c < n_main:
                              sz = 127
                              rhs = wnh[c // kqh][0:127,
                                                  (c % kqh) * 512 : (c % kqh + 1) * 512]
                          else:
                              sz = ktail
                              rhs = wtail[:]
                          nc.tensor.matmul(lg[:], xtb[bt][0:sz, c * 128 : c * 128 + 128],
                                           rhs, start=(c == 0), stop=(c == n_ck - 1))
                      c_ = bt * jt + p
                      nc.vector.reduce_max(mxall[:, c_ : c_ + 1], lg[:],
                                           axis=mybir.AxisListType.X)
                      nmx = smp.tile([128, 1], F32, name=f"nmx{p}_{bt}", tag="nmx")
                      nc.vector.tensor_scalar_mul(nmx[:], mxall[:, c_ : c_ + 1], -1.0)
                      expt = smp.tile([128, 512], F16, name=f"exp{p}_{bt}", tag="exp",
                                      bufs=3)
                      nc.scalar.activation(expt[:], lg[:], AF.Exp, bias=nmx[:])
                      nc.scalar.dma_start(
                          out_d[bt * 128 : (bt + 1) * 128, p * 512 : (p + 1) * 512],
                          expt[:])
              nc.scalar.dma_start(mxo_d[:], mxall[:])

    nc.compile()
    meta = dict(B=B, N=N, M=M, n_cores=n_cores, n_shard=n_shard, b_shard=b_shard,
                plan=plan, kt_all=kt_all, nck_all=nck_all, bt_all=bt_all, jt=jt,
                agmap=agmap)
    return nc, meta


def tc_ctx(nc):
    return tile.TileContext(nc, pool_alloc_mode="queue")


_CACHE = {}


def _get_program(B, N, M, n_cores):
    key = (B, N, M, n_cores)
    if key not in _CACHE:
        _CACHE[key] = build_program(B, N, M, n_cores)
    return _CACHE[key]


def make_in_maps(meta, consts, X, weight):
    n_cores, n_shard, b_shard = meta["n_cores"], meta["n_shard"], meta["b_shard"]
    plan, nck_all, bt_all = meta["plan"], meta["nck_all"], meta["bt_all"]
    nt = len(plan)
    M = meta["M"]
    agmap = meta["agmap"]
    # gathered-global row g = n_shard*r + l holds original W row n_shard*r + agmap[l]
    oidx = np.concatenate([c * n_shard + np.asarray(agmap) for c in range(n_cores)])
    XTp = np.ascontiguousarray(X.T[oidx]).astype(np.float16)  # [N, B]
    base = {k: v for k, v in consts.items() if not k.startswith("_")}
    selb_base, gamma, b1 = consts["_selb"], consts["_gamma"], consts["_b1"]
    colsum = weight.sum(axis=0, dtype=np.float64).astype(np.float32)
    rowsum = weight.sum(axis=1, dtype=np.float64).astype(np.float32)
    in_maps = []
    for c in range(n_cores):
        m = dict(base)
        wt = np.zeros((nt, 128, M), dtype=np.float32)
        acc = 0
        for ti, rows in enumerate(plan):
            wt[ti, 0:rows, :] = weight[c * n_shard + acc : c * n_shard + acc + rows, :]
            wt[ti, 126, :] = colsum
            wt[ti, 127, :] = 1.0
            st = selb_base[rows].copy()
            rs = rowsum[c * n_shard + acc : c * n_shard + acc + rows]
            st[127, :] = (rs[:, None] * gamma[None, :] + b1[None, :]).reshape(-1)
            m[f"selbT{ti}"] = st.astype(np.float16)
            acc += rows
        m["wsh"] = wt.astype(np.float16)
        slab = XTp[:, c * b_shard : (c + 1) * b_shard]  # [N, b_shard] gathered order
        N_ = slab.shape[0]
        xtc = np.zeros((bt_all, 128, nck_all * 128), dtype=np.float16)
        for ck in range(nck_all):
            r0 = ck * 127
            sz = min(127, N_ - r0)
            blk = slab[r0 : r0 + sz].reshape(sz, bt_all, 128)  # [sz, bt, 128]
            xtc[:, 0:sz, ck * 128 : (ck + 1) * 128] = blk.transpose(1, 0, 2)
        m["xtc"] = xtc
        in_maps.append(m)
    return in_maps


def combine_outputs(core_results, meta):
    """core_results: list of {'out': [b_shard, M] f16 exp(l - max_p) per
    512-col pass slice, 'mxo': [128, bt_all*jt] f32 per-pass row maxes}."""
    bt_all, jt, M = meta["bt_all"], meta["jt"], meta["M"]
    blocks = []
    for r in core_results:
        o = np.asarray(r["out"], dtype=np.float32).reshape(bt_all, 128, jt, 512)
        mx = np.asarray(r["mxo"], dtype=np.float32).reshape(128, bt_all, jt)
        mx = mx.transpose(1, 0, 2)  # [bt, 128, jt]
        scale = np.exp(mx - mx.max(axis=2, keepdims=True))
        o = o * scale[:, :, :, None]
        o = o.reshape(bt_all * 128, M)
        o /= o.sum(axis=1, keepdims=True)
        blocks.append(o)
    return np.concatenate(blocks, axis=0)


def run(X, weight, W1, b1, W2, b2, W3, b3, n_cores=8, trace=False, **hw_kwargs):
    X = np.asarray(X, dtype=np.float32)
    weight = np.asarray(weight, dtype=np.float32)
    B, N = X.shape
    M = weight.shape[1]
    nc, meta = _get_program(B, N, M, n_cores)
    consts = build_consts(np.asarray(W1, np.float32), np.asarray(b1, np.float32),
                          np.asarray(W2, np.float32), np.asarray(b2, np.float32),
                          np.asarray(W3, np.float32), N, M, meta["n_shard"])
    in_maps = make_in_maps(meta, consts, X, weight)
    res = bass_utils.run_bass_kernel_spmd(nc, in_maps, core_ids=list(range(n_cores)),
                                          trace=trace, **hw_kwargs)
    out = combine_outputs([res.results[c] for c in range(n_cores)], meta)
    return out, res


def kernel(X, weight, W1, b1, W2, b2, W3, b3):
    out, _ = run(X, weight, W1, b1, W2, b2, W3, b3)
    return out


# revision 28
# speedup vs baseline: 1.5073x; 1.5073x over previous
"""MetaNCA Trainium2 kernel: out = softmax(X @ (W + MLP_percell(W))).

Strategy (8 NeuronCores, SPMD, fp16 matmuls — 1 cyc/row on HW vs 2 for f32r):
  - W row-sharded (256 rows/core) as 126-row tiles; partition 126/127 of each
    tile hold the colsum row / ones row so the whole first MLP layer is ONE
    K=128 matmul per sub-chunk. Global colsum/rowsum baked in on host.
  - MLP (3->10->10->1) as block-diag fp16 matmuls, 12 rows/chain, software
    pipelined 3 deep, column-slice outer (j of 512 cols); each finished slice
    is staged and AllGathered immediately (4 column AGs overlap MLP+phase 3).
    AG outputs land in Shared-addr-space DRAM (fast HBM-HBM collective path).
  - Phase 3: per column pass p, per X row-block bt, accumulate [128,512]
    logits over 16 k-tiles in one PSUM bank (8 banks = 8 row-blocks in
    flight); then ONLINE softmax: rowwise max of the pass slice, exp(l-max_p)
    written fp16 straight from PSUM, DMA'd out. Per-pass maxes are returned
    in a side tensor; host rescales slices by exp(max_p - max_row) and
    normalizes. X^T is host-pre-arranged fp16 per-core as [bt][p][kt*128].
"""

import os
import sys

import numpy as np

for _p in ("/opt/trn_rl_repo", "/root/.axon_site/_ro/trn_rl_repo"):
    if os.path.isdir(_p) and _p not in sys.path:
        sys.path.insert(0, _p)

import concourse.bass as bass  # noqa: E402
import concourse.tile as tile  # noqa: E402
from concourse import bacc, bass_utils, mybir  # noqa: E402

F32 = mybir.dt.float32
F16 = mybir.dt.float16
BF16 = mybir.dt.bfloat16
F32R = mybir.dt.float32r
AF = mybir.ActivationFunctionType
Alu = mybir.AluOpType
H = 10
RW = 126  # real W rows per tile (126/127 = colsum/ones)
# tile-scheduler pseudo-time hints (ms) for phase-3 weight loads
P3_W0 = 0.055
P3_WP = 0.020


def _tile_plan(n_shard):
    plan = []
    r = 0
    while r + RW <= n_shard:
        plan.append(RW)
        r += RW
    if r < n_shard:
        plan.append(n_shard - r)
    return plan


def _subchunks(rows):
    subs = []
    r = 0
    while r < rows:
        g = min(12, rows - r)
        subs.append((r, g))
        r += g
    return subs


def _agmap(n_shard, plan):
    """ag_in row -> local shard row: tile boundaries placed so [0:128) is
    covered by tiles {0, last} and [128:256) by {1, last}."""
    assert n_shard == 256 and plan == [126, 126, 4]
    t0 = list(range(0, 126))
    t1 = list(range(126, 252))
    t2 = list(range(252, 256))
    return t0 + t2[0:2] + t1 + t2[2:4]


def build_consts(W1, b1, W2, b2, W3, n, m, n_shard):
    alpha = (W1[0] - W1[1] / np.float32(n - 1) - W1[2] / np.float32(m - 1)).astype(np.float32)
    beta = (W1[1] / np.float32(n - 1)).astype(np.float32)
    gamma = (W1[2] / np.float32(m - 1)).astype(np.float32)
    plan = _tile_plan(n_shard)

    def selb(rows):
        cols = rows * H
        t = np.zeros((128, cols), dtype=np.float32)
        for r in range(rows):
            t[r, r * H : (r + 1) * H] = alpha
        t[126, :] = np.tile(beta, rows)
        return t

    def w3sc(rows):
        subs = _subchunks(rows)
        t = np.zeros((120, len(subs) * 128), dtype=np.float32)
        for s, (r0, g_) in enumerate(subs):
            for g in range(g_):
                t[g * H : (g + 1) * H, s * 128 + r0 + g] = W3[:, 0]
        return t

    def blkdiag(mat, g_):
        out = np.zeros((g_ * mat.shape[0], g_ * mat.shape[1]), dtype=np.float32)
        for g in range(g_):
            out[g * mat.shape[0] : (g + 1) * mat.shape[0],
                g * mat.shape[1] : (g + 1) * mat.shape[1]] = mat
        return out

    gset = sorted({g for rows in set(plan) for _, g in _subchunks(rows)})
    c = {}
    for rows in sorted(set(plan)):
        c[f"w3sc{rows}"] = w3sc(rows).astype(np.float16)
    for g_ in gset:
        c[f"w2b{g_}"] = blkdiag(W2, g_).astype(np.float16)
        c[f"b2t{g_}"] = np.tile(b2, g_)[:, None].astype(np.float32)
    c["_selb"] = {rows: selb(rows) for rows in sorted(set(plan))}
    c["_gamma"] = gamma
    c["_b1"] = b1
    return c


def build_program(B, N, M, n_cores):
    n_shard = N // n_cores
    b_shard = B // n_cores
    plan = _tile_plan(n_shard)
    nt = len(plan)
    kt_all = N // 128
    nck_all = N // 127 + (1 if N % 127 else 0)  # phase-3 K-chunks of <=127
    bt_all = b_shard // 128
    jt = M // 512
    tiles_order = [0, nt - 1] + list(range(1, nt - 1)) if nt > 2 else list(range(nt))

    nc = bacc.Bacc("TRN2", target_bir_lowering=False, debug=False, num_devices=n_cores)

    d = {}
    def din(name, shape, dt):
        d[name] = nc.dram_tensor(name, list(shape), dt, kind="ExternalInput").ap()
    din("wsh", (nt, 128, M), F16)              # W shard tiles; row126=colsum, row127=1
    din("xtc", (bt_all, 128, kt_all * 128), F16)  # X^T, [bt][k-in-kt][kt*128+b]
    for ti, rows in enumerate(plan):
        din(f"selbT{ti}", (128, rows * H), F16)   # alpha diag, row126=beta, row127=r13
    for rows in sorted(set(plan)):
        din(f"w3sc{rows}", (120, len(_subchunks(rows)) * 128), F16)
    gset = sorted({g for rows in set(plan) for _, g in _subchunks(rows)})
    for g_ in gset:
        din(f"w2b{g_}", (g_ * H, g_ * H), F16)
        din(f"b2t{g_}", (g_ * H, 1), F32)
    out_d = nc.dram_tensor("out", [b_shard, M], F16, kind="ExternalOutput").ap()
    mxo_d = nc.dram_tensor("mxo", [128, bt_all * jt], F32, kind="ExternalOutput").ap()

    rg = [list(range(n_cores))]
    agmap = _agmap(n_shard, plan) if n_shard == 256 else list(range(n_shard))

    # contiguous runs (ag_row, tile, local_row, count) of the agmap
    tile_base = []
    acc = 0
    for rows in plan:
        tile_base.append(acc)
        acc += rows
    def tile_of(shard_row):
        for t in range(len(plan) - 1, -1, -1):
            if shard_row >= tile_base[t]:
                return t, shard_row - tile_base[t]
        raise AssertionError
    def ag_runs():
        runs = []
        i = 0
        while i < n_shard:
            t0_, lr0 = tile_of(agmap[i])
            j = i
            while j + 1 < n_shard:
                t1_, lr1 = tile_of(agmap[j + 1])
                if t1_ != t0_ or lr1 != lr0 + (j + 1 - i):
                    break
                j += 1
            runs.append((i, t0_, lr0, j - i + 1))
            i = j + 1
        return runs
    runs = ag_runs()

    with tc_ctx(nc) as tc:
      with tc.tile_pool(name="dram", bufs=1, space="DRAM") as dram:
        ag_in = [dram.tile([n_shard, 512], F16, name=f"ag_in{j}") for j in range(jt)]
        ag_space = "Shared" if n_cores > 4 else "Local"
        agS = [dram.tile([n_cores * n_shard, 512], F16, name=f"agS{j}",
                         addr_space=ag_space) for j in range(jt)]
        with tc.tile_pool(name="xp", bufs=1) as xp:
          with tc.tile_pool(name="cp", bufs=1) as cp, \
               tc.tile_pool(name="wp", bufs=1) as wp:
            def load(pool, name, dram_ap, shape, dt, eng=None):
                t = pool.tile(shape, dt, name=name)
                (eng or nc.scalar).dma_start(t[:], dram_ap[:])
                return t

            # MLP-critical loads on sync, ordered by first use; scalar ring
            # stays clear so the relu stream starts immediately
            w_t, selbw = [None] * nt, [None] * nt
            w3_t, w2b_t, b2t_t = {}, {}, {}
            def load_tile_pair(ti, eng, sliced=False):
                rows = plan[ti]
                t = wp.tile([128, M], F16, name=f"w_t{ti}")
                st = wp.tile([128, rows * H], F16, name=f"selbw{ti}")
                eng.dma_start(st[:], d[f"selbT{ti}"][:])
                if sliced:
                    # j-slice the first tile's load so mm1 of (j=0, ti) can
                    # start as soon as its first 512 columns land
                    for j in range(jt):
                        sl = slice(j * 512, (j + 1) * 512)
                        eng.dma_start(t[:, sl], d["wsh"][ti, :, sl])
                else:
                    eng.dma_start(t[:], d["wsh"][ti])
                w_t[ti], selbw[ti] = t, st
            load_tile_pair(tiles_order[0], nc.sync, sliced=True)
            load_tile_pair(tiles_order[-1], nc.scalar, sliced=True)
            for g_ in gset:
                w2b_t[g_] = load(cp, f"w2b_t{g_}", d[f"w2b{g_}"],
                                 [g_ * H, g_ * H], F16, eng=nc.sync)
                b2t_t[g_] = load(cp, f"b2t_t{g_}", d[f"b2t{g_}"], [g_ * H, 1], F32,
                                 eng=nc.sync)
            for rows in sorted(set(plan), reverse=True):
                w3_t[rows] = load(cp, f"w3_t{rows}", d[f"w3sc{rows}"],
                                  [120, len(_subchunks(rows)) * 128], F16,
                                  eng=nc.sync)
            for ti in tiles_order[1:-1]:
                load_tile_pair(ti, nc.sync)
            # X prefetch after the critical loads (sync ring; needed ~t+60us,
            # must NOT sit on the scalar ring ahead of the MLP relu stream)
            xtb = []
            for bt in range(bt_all):
                t = xp.tile([128, kt_all * 128], F16, name=f"xtb{bt}")
                nc.sync.dma_start(t[:], d["xtc"][bt])
                xtb.append(t)

            with tc.tile_pool(name="p1", bufs=1) as p1:
                # warm exp + relu activation tables before the relu stream
                # needs them (each cold table load costs ~2.7us on scalar)
                wdum = p1.tile([1, 8], F32, name="wdum")
                nc.vector.memset(wdum[:], 0.0)
                nc.scalar.activation(wdum[:], wdum[:], AF.Exp)
                nc.scalar.activation(wdum[:], wdum[:], AF.Relu)

            # ---- phase 2: MLP, column-slice outer; AG per column slice
            with tc.tile_pool(name="nwp", bufs=1) as nwp, \
                 tc.tile_pool(name="hp", bufs=3) as hp, \
                 tc.tile_pool(name="p2ps", bufs=1, space="PSUM") as p2ps:
                chains = []
                for j in range(jt):
                    for ti in tiles_order:
                        rows = plan[ti]
                        for s, (r0, g_) in enumerate(_subchunks(rows)):
                            chains.append((ti, j, s, r0, g_, rows))
                nC = len(chains)
                state = {}
                nw_t = {}
                # greedy scalar/vector balance for the PSUM-source elementwise
                # stream (both engines are ~1x from PSUM; vector also owns the
                # nw adds)
                ew_ns = {"s": 0.0, "v": 0.0}

                def pick_ew():
                    k = "s" if ew_ns["s"] <= ew_ns["v"] else "v"
                    ew_ns[k] += 690.0
                    return k

                def emit_mm1(c, idx):
                    ti, j, s, r0, g_, rows = c
                    sl = slice(j * 512, (j + 1) * 512)
                    Mh = g_ * H
                    ps1 = p2ps.tile([120, 512], F32, name=f"ps1_{ti}_{j}_{s}", tag="ps1", bufs=3)
                    nc.tensor.matmul(ps1[0:Mh, :], selbw[ti][:, r0 * H : r0 * H + Mh],
                                     w_t[ti][:, sl], start=True, stop=True)
                    h1 = hp.tile([120, 512], F16, name=f"h1_{ti}_{j}_{s}", tag="h1")
                    if pick_ew() == "s":
                        nc.scalar.activation(h1[0:Mh, :], ps1[0:Mh, :], AF.Relu)
                    else:
                        nc.vector.tensor_scalar(h1[0:Mh, :], ps1[0:Mh, :], 0.0,
                                                0.0, op0=Alu.max, op1=Alu.bypass)
                    state[c] = (ps1, h1)

                def emit_mm2(c, idx):
                    ti, j, s, r0, g_, rows = c
                    Mh = g_ * H
                    _, h1 = state[c]
                    ps2 = p2ps.tile([120, 512], F32, name=f"ps2_{ti}_{j}_{s}", tag="ps2", bufs=3)
                    nc.tensor.matmul(ps2[0:Mh, :], w2b_t[g_][:], h1[0:Mh, :], start=True, stop=True)
                    h2 = hp.tile([120, 512], F16, name=f"h2_{ti}_{j}_{s}", tag="h2")
                    if pick_ew() == "v":
                        nc.vector.tensor_scalar(h2[0:Mh, :], ps2[0:Mh, :], b2t_t[g_][0:Mh, :],
                                                0.0, op0=Alu.add, op1=Alu.max)
                    else:
                        nc.scalar.activation(h2[0:Mh, :], ps2[0:Mh, :], AF.Relu,
                                             bias=b2t_t[g_][0:Mh, :])
                    state[c] = (state[c][0], state[c][1], ps2, h2)

                def emit_mm3(c):
                    ti, j, s, r0, g_, rows = c
                    sl = slice(j * 512, (j + 1) * 512)
                    Mh = g_ * H
                    h2 = state.pop(c)[3]
                    subs = _subchunks(rows)
                    key = (ti, j)
                    if key not in upd_ps:
                        upd_ps[key] = p2ps.tile([128, 512], F32, name=f"upd_{ti}_{j}",
                                                tag="upd", bufs=2)
                    nc.tensor.matmul(upd_ps[key][:], w3_t[rows][0:Mh, s * 128 : (s + 1) * 128],
                                     h2[0:Mh, :], start=(s == 0), stop=(s == len(subs) - 1))
                    if s == len(subs) - 1:
                        if ti not in nw_t:
                            nw_t[ti] = nwp.tile([128, M], F16, name=f"nw_t{ti}", tag=f"nw{ti}")
                        nc.vector.tensor_tensor(nw_t[ti][0:rows, sl], upd_ps[key][0:rows, :],
                                                w_t[ti][0:rows, sl], op=Alu.add)
                        ew_ns["v"] += 690.0
                        del upd_ps[key]
                        # stage this tile's rows of column slice j into ag_in[j]
                        for (agr, ti2, lr, cnt) in runs:
                            if ti2 != ti:
                                continue
                            nc.gpsimd.dma_start(ag_in[j][agr : agr + cnt, :],
                                              nw_t[ti][lr : lr + cnt, sl])
                        done_tiles[j].add(ti)
                        if len(done_tiles[j]) == nt and not agd.get(j):
                            agd[j] = True
                            nc.gpsimd.collective_compute(
                                "AllGather", Alu.bypass, ins=[ag_in[j].opt()],
                                outs=[agS[j].opt()], replica_groups=rg)

                upd_ps, agd = {}, {}
                done_tiles = {j: set() for j in range(jt)}
                D2, D3 = 2, 4
                for i in range(nC + D3):
                    if i < nC:
                        emit_mm1(chains[i], i)
                    if 0 <= i - D2 < nC:
                        emit_mm2(chains[i - D2], i)
                    if 0 <= i - D3 < nC:
                        emit_mm3(chains[i - D3])

          # ---- phase 3: per column pass, accumulate logits over 16 K-tiles.
          # Two row-blocks' accumulation chains are INTERLEAVED so consecutive
          # matmuls hit different PSUM banks: back-to-back writes into one
          # bank serialize the ~128-cycle output drain (263ns/mm observed);
          # alternating banks overlaps drain with the other chain's fill
          # (~220ns/mm). Online softmax per (pass, row-block).
          assert kt_all % 4 == 0 and bt_all % 2 == 0
          kth = kt_all // 4
          with tc.tile_pool(name="wnp", bufs=1) as wnp, \
               tc.tile_pool(name="smp", bufs=2) as smp, \
               tc.tile_pool(name="mxp", bufs=1) as mxp, \
               tc.tile_pool(name="p3ps", bufs=1, space="PSUM") as p3ps:
              mxall = mxp.tile([128, bt_all * jt], F32, name="mxall")
              for p in range(jt):
                  tc.tile_set_cur_wait(P3_W0 + P3_WP * p)
                  # quarter loads on sync+gpsimd rings (parallel transfers);
                  # the scalar ring is reserved for the exp/out stream so
                  # matmul-feeding loads never queue behind it
                  wnh = []
                  for hf in range(4):
                      t16 = wnp.tile([128, kth * 512], F16, name=f"wn16_{p}_{hf}",
                                     tag="wn16", bufs=8)
                      src = agS[p].rearrange("(t q) m -> q t m", q=128)[
                          :, hf * kth : (hf + 1) * kth, :]
                      eng = nc.sync if hf % 2 == 0 else nc.gpsimd
                      eng.dma_start(t16[:].rearrange("q (t m) -> q t m", m=512), src)
                      wnh.append(t16)
                  for bt0 in range(0, bt_all, 2):
                      lgA = p3ps.tile([128, 512], F32, name=f"lg{p}_{bt0}", tag="lg",
                                      bufs=min(8, bt_all))
                      lgB = p3ps.tile([128, 512], F32, name=f"lg{p}_{bt0 + 1}", tag="lg",
                                      bufs=min(8, bt_all))
                      for kt in range(kt_all):
                          wt_ = wnh[kt // kth]
                          ksl = slice((kt % kth) * 512, (kt % kth) * 512 + 512)
                          for lg, bt in ((lgA, bt0), (lgB, bt0 + 1)):
                              nc.tensor.matmul(lg[:], xtb[bt][:, kt * 128 : (kt + 1) * 128],
                                               wt_[:, ksl], start=(kt == 0),
                                               stop=(kt == kt_all - 1))
                      for lg, bt in ((lgA, bt0), (lgB, bt0 + 1)):
                          c_ = bt * jt + p
                          nc.vector.reduce_max(mxall[:, c_ : c_ + 1], lg[:],
                                               axis=mybir.AxisListType.X)
                          nmx = smp.tile([128, 1], F32, name=f"nmx{p}_{bt}", tag="nmx")
                          nc.vector.tensor_scalar_mul(nmx[:], mxall[:, c_ : c_ + 1], -1.0)
                          expt = smp.tile([128, 512], F16, name=f"exp{p}_{bt}", tag="exp",
                                          bufs=3)
                          nc.scalar.activation(expt[:], lg[:], AF.Exp, bias=nmx[:])
                          nc.scalar.dma_start(
                              out_d[bt * 128 : (bt + 1) * 128, p * 512 : (p + 1) * 512],
                              expt[:])
              nc.scalar.dma_start(mxo_d[:], mxall[:])

    nc.compile()
    meta = dict(B=B, N=N, M=M, n_cores=n_cores, n_shard=n_shard, b_shard=b_shard,
                plan=plan, kt_all=kt_all, nck_all=nck_all, bt_all=bt_all, jt=jt,
                agmap=agmap)
    return nc, meta


def tc_ctx(nc):
    return tile.TileContext(nc, pool_alloc_mode="queue")


_CACHE = {}


def _get_program(B, N, M, n_cores):
    key = (B, N, M, n_cores)
    if key not in _CACHE:
        _CACHE[key] = build_program(B, N, M, n_cores)
    return _CACHE[key]


def make_in_maps(meta, consts, X, weight):
    n_cores, n_shard, b_shard = meta["n_cores"], meta["n_shard"], meta["b_shard"]
    plan, kt_all, bt_all = meta["plan"], meta["kt_all"], meta["bt_all"]
    nt = len(plan)
    M = meta["M"]
    agmap = meta["agmap"]
    # gathered-global row g = n_shard*r + l holds original W row n_shard*r + agmap[l]
    oidx = np.concatenate([c * n_shard + np.asarray(agmap) for c in range(n_cores)])
    XTp = np.ascontiguousarray(X.T[oidx]).astype(np.float16)  # [N, B]
    base = {k: v for k, v in consts.items() if not k.startswith("_")}
    selb_base, gamma, b1 = consts["_selb"], consts["_gamma"], consts["_b1"]
    colsum = weight.sum(axis=0, dtype=np.float64).astype(np.float32)
    rowsum = weight.sum(axis=1, dtype=np.float64).astype(np.float32)
    in_maps = []
    for c in range(n_cores):
        m = dict(base)
        wt = np.zeros((nt, 128, M), dtype=np.float32)
        acc = 0
        for ti, rows in enumerate(plan):
            wt[ti, 0:rows, :] = weight[c * n_shard + acc : c * n_shard + acc + rows, :]
            wt[ti, 126, :] = colsum
            wt[ti, 127, :] = 1.0
            st = selb_base[rows].copy()
            rs = rowsum[c * n_shard + acc : c * n_shard + acc + rows]
            st[127, :] = (rs[:, None] * gamma[None, :] + b1[None, :]).reshape(-1)
            m[f"selbT{ti}"] = st.astype(np.float16)
            acc += rows
        m["wsh"] = wt.astype(np.float16)
        slab = XTp[:, c * b_shard : (c + 1) * b_shard]  # [N, b_shard] gathered order
        m["xtc"] = np.ascontiguousarray(
            slab.reshape(kt_all, 128, bt_all, 128).transpose(2, 1, 0, 3)
            .reshape(bt_all, 128, kt_all * 128))
        in_maps.append(m)
    return in_maps


def combine_outputs(core_results, meta):
    """core_results: list of {'out': [b_shard, M] f16 exp(l - max_p) per
    512-col pass slice, 'mxo': [128, bt_all*jt] f32 per-pass row maxes}."""
    bt_all, jt, M = meta["bt_all"], meta["jt"], meta["M"]
    blocks = []
    for r in core_results:
        o = np.asarray(r["out"], dtype=np.float32).reshape(bt_all, 128, jt, 512)
        mx = np.asarray(r["mxo"], dtype=np.float32).reshape(128, bt_all, jt)
        mx = mx.transpose(1, 0, 2)  # [bt, 128, jt]
        scale = np.exp(mx - mx.max(axis=2, keepdims=True))
        o = o * scale[:, :, :, None]
        o = o.reshape(bt_all * 128, M)
        o /= o.sum(axis=1, keepdims=True)
        blocks.append(o)
    return np.concatenate(blocks, axis=0)


def run(X, weight, W1, b1, W2, b2, W3, b3, n_cores=8, trace=False, **hw_kwargs):
    X = np.asarray(X, dtype=np.float32)
    weight = np.asarray(weight, dtype=np.float32)
    B, N = X.shape
    M = weight.shape[1]
    nc, meta = _get_program(B, N, M, n_cores)
    consts = build_consts(np.asarray(W1, np.float32), np.asarray(b1, np.float32),
                          np.asarray(W2, np.float32), np.asarray(b2, np.float32),
                          np.asarray(W3, np.float32), N, M, meta["n_shard"])
    in_maps = make_in_maps(meta, consts, X, weight)
    res = bass_utils.run_bass_kernel_spmd(nc, in_maps, core_ids=list(range(n_cores)),
                                          trace=trace, **hw_kwargs)
    out = combine_outputs([res.results[c] for c in range(n_cores)], meta)
    return out, res


def kernel(X, weight, W1, b1, W2, b2, W3, b3):
    out, _ = run(X, weight, W1, b1, W2, b2, W3, b3)
    return out
